# revision 53
# baseline (speedup 1.0000x reference)
"""AdaAttN 3D stylizer — distributed Bass kernel for 8 TRN2 NeuronCores.

Sharding: data-parallel over batch (2) x sequence-parallel over content
tokens N (4-way) -> 8 cores. Core c handles batch b=c//4, token slice
[(c%4)*2048, (c%4+1)*2048). Style tokens (M=4096) are fully replicated
per core, so the attention softmax / mean / var reductions over M are
local. The only cross-core reduction is the instance-norm statistics of
q0/c0 over the full N=8192 axis: a single 4KB AllReduce per batch group
([[0..3],[4..7]]), overlapped with the style-side projections.

Compute dtype: bf16 on the PE (fp32 PSUM accumulation), fp32 for all
statistics and the softmax denominators.
"""

import sys
import types

import numpy as np
import ml_dtypes

if "/opt/trn_rl_repo" not in sys.path:
    sys.path.insert(0, "/opt/trn_rl_repo")


def _install_ntff_shim():
    """Make run_bass_kernel_spmd(trace=True) degrade gracefully (or work,
    when the axon profiling lib is present) even if antenv.axon_hooks is
    not importable in this image."""
    try:
        import antenv.axon_hooks  # noqa: F401
        return
    except ImportError:
        pass
    mod = types.ModuleType("antenv.axon_hooks")
    mod._hook = None

    def set_axon_ntff_profile_hook(h):
        mod._hook = h

    def get_axon_ntff_profile_hook():
        return mod._hook

    mod.set_axon_ntff_profile_hook = set_axon_ntff_profile_hook
    mod.get_axon_ntff_profile_hook = get_axon_ntff_profile_hook
    sys.modules["antenv.axon_hooks"] = mod
    try:
        import antenv
        antenv.axon_hooks = mod
    except ImportError:
        pass
    try:
        from trn_agent_boot.trn_boot import _ntff_profile_via_ctypes
        set_axon_ntff_profile_hook(_ntff_profile_via_ctypes("/opt/axon/libaxon_pjrt.so"))
    except Exception:
        pass


_install_ntff_shim()

import concourse.bacc as bacc  # noqa: E402
import concourse.mybir as mybir  # noqa: E402
import concourse.tile as tile  # noqa: E402
from concourse.bass_utils import run_bass_kernel_spmd  # noqa: E402

F32 = mybir.dt.float32
BF16 = mybir.dt.bfloat16
AF = mybir.ActivationFunctionType
ALU = mybir.AluOpType
AXX = mybir.AxisListType.X

BS, C, N = 2, 512, 8192
D, M = 256, 4096
NL = N // 4          # tokens per core
NF = 512             # free-dim chunk (one PSUM bank of f32)
NCH = NL // NF       # chunks per core
MT = M // 128        # style tiles of 128
EPS = 1e-5

# bias-pack column indices ([128, 18] f32)
B_QZ1, B_QZ2, B_VZ1, B_VZ2, B_QE, B_KE, B_VU1, B_VU2 = 0, 2, 4, 6, 8, 10, 12, 14

_nc_cache = {}


def _lrelu_dve(nc, out_ap, in_ap):
    # lrelu(x) = max(0.2*x, x)
    nc.vector.scalar_tensor_tensor(out_ap, in_ap, 0.2, in_ap, ALU.mult, ALU.max)


def _build_nc():
    nc = bacc.Bacc("TRN2", target_bir_lowering=False, debug=False)

    x_d = nc.dram_tensor("x", [C, NL], BF16, kind="ExternalInput")
    sty_d = nc.dram_tensor("sty", [D, M], BF16, kind="ExternalInput")
    wq1_d = nc.dram_tensor("wq1", [C, D], BF16, kind="ExternalInput")
    wq2_d = nc.dram_tensor("wq2", [D, D], BF16, kind="ExternalInput")
    wv1_d = nc.dram_tensor("wv1", [C, D], BF16, kind="ExternalInput")
    wv2_d = nc.dram_tensor("wv2", [D, D], BF16, kind="ExternalInput")
    wqe_d = nc.dram_tensor("wqe", [D, D], BF16, kind="ExternalInput")
    wke_d = nc.dram_tensor("wke", [D, D], BF16, kind="ExternalInput")
    wse_d = nc.dram_tensor("wse", [D + 1, D], BF16, kind="ExternalInput")
    wu1_d = nc.dram_tensor("wu1", [D, D], BF16, kind="ExternalInput")
    wu2_d = nc.dram_tensor("wu2", [D, C], BF16, kind="ExternalInput")
    bias_d = nc.dram_tensor("bias", [128, 18], F32, kind="ExternalInput")
    out_d = nc.dram_tensor("out", [C, NL], F32, kind="ExternalOutput")

    with tile.TileContext(nc) as tc:
        with tc.tile_pool(name="wp", bufs=1) as wp, \
             tc.tile_pool(name="bigp", bufs=1) as bigp, \
             tc.tile_pool(name="workp", bufs=2) as workp, \
             tc.tile_pool(name="dramp", bufs=1, space="DRAM") as dp:

            # ---- weights / constants ----
            def wload(name, dram, kparts, nfree):
                t = wp.tile([128, kparts, nfree], BF16, name=name, tag=name)
                for ki in range(kparts):
                    nc.sync.dma_start(t[:, ki, :], dram[ki * 128:(ki + 1) * 128, :])
                return t

            wq1_sb = wload("wq1_sb", wq1_d, 4, 256)
            bias_sb = wp.tile([128, 18], F32, name="bias_sb", tag="bias_sb")
            nc.sync.dma_start(bias_sb[:], bias_d[:])
            xb = [bigp.tile([128, NL], BF16, name=f"xb{ci}", tag=f"xb{ci}")
                  for ci in range(4)]
            for ci in range(4):
                nc.sync.dma_start(xb[ci][:], x_d[ci * 128:(ci + 1) * 128, :])
            wv1_sb = wload("wv1_sb", wv1_d, 4, 256)
            wq2_sb = wload("wq2_sb", wq2_d, 2, 256)
            wv2_sb = wload("wv2_sb", wv2_d, 2, 256)
            wqe_sb = wload("wqe_sb", wqe_d, 2, 256)
            wke_sb = wload("wke_sb", wke_d, 2, 256)
            wu1_sb = wload("wu1_sb", wu1_d, 2, 256)
            wu2_sb = wload("wu2_sb", wu2_d, 2, 512)
            wse_sb = wp.tile([128, 3, 256], BF16, name="wse_sb", tag="wse_sb")
            for ki in range(2):
                nc.sync.dma_start(wse_sb[:, ki, :], wse_d[ki * 128:(ki + 1) * 128, :])
            nc.sync.dma_start(wse_sb[0:1, 2, :], wse_d[256:257, :])
            ones_bf = wp.tile([1, 128], BF16, name="ones_bf", tag="ones_bf")
            nc.vector.memset(ones_bf[:], 1.0)
            onecol_bf = wp.tile([128, 1], BF16, name="onecol_bf", tag="onecol_bf")
            nc.vector.memset(onecol_bf[:], 1.0)
            ones_f32 = wp.tile([1, 128], F32, name="ones_f32", tag="ones_f32")
            nc.vector.memset(ones_f32[:], 1.0)
            eps_sb = wp.tile([128, 1], F32, name="eps_sb", tag="eps_sb")
            nc.vector.memset(eps_sb[:], EPS)

            # startup barrier: absorb cross-core NEFF-launch skew here (PE is
            # loading weights anyway) so the real stats collectives later see
            # aligned peers instead of paying the skew on the critical path
            sync_in = dp.tile([1, 1], F32, name="sync_in")
            sync_out = dp.tile([8, 1], F32, name="sync_out")
            sync_sb = wp.tile([1, 1], F32, name="sync_sb", tag="sync_sb")
            nc.vector.memset(sync_sb[:], 0.0)
            nc.gpsimd.dma_start(sync_in[:], sync_sb[:])
            nc.gpsimd.collective_compute(
                "AllGather", ALU.bypass,
                replica_groups=[[0, 1, 2, 3, 4, 5, 6, 7]],
                ins=[sync_in[:].opt()], outs=[sync_out[:].opt()])

            with tc.tile_pool(name="ppA", bufs=8, space="PSUM") as ppA:
                # ---- phase 1: zipper MLPs + partial stats (x was host-cast to
                # bf16 and loaded right after wq1 on the DMA queue) ----
                parts = bigp.tile([128, 32], F32, name="parts", tag="parts")

                def zipper(w1_sb, w2_sb, b1c, b2c, pfx, sum_base, sq_base):
                    # h = lrelu(w1.T @ x + b1)  -> bf16 [2][128, NL]
                    h = [bigp.tile([128, NL], BF16, name=f"{pfx}h{oj}", tag=f"h1_{oj}")
                         for oj in range(2)]
                    for jn in range(NCH):
                        for oj in range(2):
                            ps = ppA.tile([128, NF], F32, name="psz", tag="mm")
                            for ki in range(4):
                                nc.tensor.matmul(
                                    ps[:], w1_sb[:, ki, oj * 128:(oj + 1) * 128],
                                    xb[ki][:, jn * NF:(jn + 1) * NF],
                                    start=(ki == 0), stop=(ki == 3))
                            hl = workp.tile([128, NF], BF16, name="hl", tag="scr512", bufs=3)
                            nc.scalar.activation(hl[:], ps[:], AF.Identity,
                                                 bias=bias_sb[:, b1c + oj:b1c + oj + 1])
                            _lrelu_dve(nc, h[oj][:, jn * NF:(jn + 1) * NF], hl[:])
                    # o = w2.T @ h + b2 -> bf16 [2][128, NL], plus sum/sumsq partials
                    o = [bigp.tile([128, NL], BF16, name=f"{pfx}o{oj}", tag=f"{pfx}o{oj}")
                         for oj in range(2)]
                    for oj in range(2):
                        for jn in range(NCH):
                            ps = ppA.tile([128, NF], F32, name="psz2", tag="mm")
                            for ki in range(2):
                                nc.tensor.matmul(
                                    ps[:], w2_sb[:, ki, oj * 128:(oj + 1) * 128],
                                    h[ki][:, jn * NF:(jn + 1) * NF],
                                    start=(ki == 0), stop=(ki == 1))
                            nc.scalar.activation(
                                o[oj][:, jn * NF:(jn + 1) * NF], ps[:], AF.Identity,
                                bias=bias_sb[:, b2c + oj:b2c + oj + 1],
                                accum_out=parts[:, sum_base + oj * 4 + jn:sum_base + oj * 4 + jn + 1])
                            sq = workp.tile([128, NF], BF16, name="sq", tag="scr512", bufs=3)
                            nc.vector.scalar_tensor_tensor(
                                sq[:], o[oj][:, jn * NF:(jn + 1) * NF], 0.0,
                                o[oj][:, jn * NF:(jn + 1) * NF], ALU.add, ALU.mult,
                                accum_out=parts[:, sq_base + oj * 4 + jn:sq_base + oj * 4 + jn + 1])
                    return o

                def stats_allreduce(pfx, base):
                    # reduce partials -> [128, 4] (sum0 sum1 sq0 sq1), AllGather
                    # within the batch group (lower floor than AllReduce), then
                    # sum the 4 ranks' contributions locally on DVE.
                    sin = workp.tile([128, 4], F32, name=f"sin_{pfx}", tag=f"sin_{pfx}",
                                     bufs=1)
                    for g in range(4):
                        nc.vector.reduce_sum(sin[:, g:g + 1],
                                             parts[:, base + g * 4:base + (g + 1) * 4],
                                             axis=AXX)
                    # collective staging DMAs go on gpsimd's queue: the
                    # result-fetch DMA blocks on the collective semaphore, and
                    # on the (in-order) sync queue it would stall every later
                    # load behind it.
                    cin = dp.tile([128, 4], F32, name=f"cc_in_{pfx}")
                    cout = dp.tile([4, 128, 4], F32, name=f"cc_out_{pfx}")
                    nc.gpsimd.dma_start(cin[:], sin[:])
                    nc.gpsimd.collective_compute(
                        "AllGather", ALU.bypass,
                        replica_groups=[[0, 1, 2, 3], [4, 5, 6, 7]],
                        ins=[cin[:].opt()], outs=[cout[:].opt()])
                    # land as [128, 4(stat), 4(rank)] so the rank axis is
                    # innermost, then reduce it
                    sg4 = workp.tile([128, 4, 4], F32, name=f"sg4_{pfx}",
                                     tag=f"sg4_{pfx}", bufs=1)
                    nc.gpsimd.dma_start(sg4[:], cout.rearrange("r p c -> p c r"))
                    sg = workp.tile([128, 4], F32, name=f"sg_{pfx}", tag=f"sg_{pfx}",
                                    bufs=1)
                    nc.vector.reduce_sum(sg[:], sg4[:], axis=AXX)
                    return sg

                q0 = zipper(wq1_sb, wq2_sb, B_QZ1, B_QZ2, "q0", 0, 8)
                stats_gq = stats_allreduce("q", 0)

                # ---- phase 2 (collective-independent): style side ----
                styb = []
                ssum = workp.tile([128, 2], F32, name="ssum", tag="ssum", bufs=1)
                ssq = workp.tile([128, 2], F32, name="ssq", tag="ssq", bufs=1)
                for oj in range(2):
                    t = bigp.tile([128, M], BF16, name=f"styb{oj}", tag=f"styb{oj}")
                    nc.sync.dma_start(t[:], sty_d[oj * 128:(oj + 1) * 128, :])
                    styb.append(t)
                for oj in range(2):
                    sqa = workp.tile([128, M], BF16, name=f"sqa{oj}", tag="sqs", bufs=1)
                    nc.scalar.activation(sqa[:], styb[oj][:], AF.Identity,
                                         accum_out=ssum[:, oj:oj + 1])
                    sqb = workp.tile([128, M], BF16, name=f"sqb{oj}", tag="sqs", bufs=1)
                    nc.vector.scalar_tensor_tensor(
                        sqb[:], styb[oj][:], 0.0, styb[oj][:], ALU.add, ALU.mult,
                        accum_out=ssq[:, oj:oj + 1])

                def norm_consts(sum_ap, sq_ap, n_axis, pfx):
                    # returns (rsig, nbias) with nbias = -mu * rsig
                    mu = workp.tile([128, 1], F32, name=f"{pfx}mu", tag=f"{pfx}mu", bufs=1)
                    nc.scalar.mul(mu[:], sum_ap, 1.0 / n_axis)
                    ex2 = workp.tile([128, 1], F32, name=f"{pfx}ex2", tag=f"{pfx}ex2", bufs=1)
                    nc.scalar.mul(ex2[:], sq_ap, 1.0 / n_axis)
                    nvar = workp.tile([128, 1], F32, name=f"{pfx}nvar", tag=f"{pfx}nvar", bufs=1)
                    # nvar = mu*mu - ex2  (= -var)
                    nc.vector.scalar_tensor_tensor(nvar[:], mu[:], mu[:], ex2[:],
                                                   ALU.mult, ALU.subtract)
                    sig = workp.tile([128, 1], F32, name=f"{pfx}sig", tag=f"{pfx}sig", bufs=1)
                    nc.scalar.activation(sig[:], nvar[:], AF.Sqrt, bias=eps_sb[:, 0:1],
                                         scale=-1.0)
                    rsig = bigp.tile([128, 1], F32, name=f"{pfx}rsig", tag=f"{pfx}rsig")
                    nc.vector.reciprocal(rsig[:], sig[:])
                    nbias = bigp.tile([128, 1], F32, name=f"{pfx}nb", tag=f"{pfx}nb")
                    nc.vector.scalar_tensor_tensor(nbias[:], mu[:], -1.0, rsig[:],
                                                   ALU.mult, ALU.mult)
                    return rsig, nbias

                sty_rs, sty_nb = [], []
                for oj in range(2):
                    rs, nb = norm_consts(ssum[:, oj:oj + 1], ssq[:, oj:oj + 1], M, f"sn{oj}")
                    sty_rs.append(rs)
                    sty_nb.append(nb)

                # kin = inorm(style) (reuses styb tags after sv is built — see below)
                # s_projT (+bias row) and sv = [sT | sT^2 | ones]
                sv = []
                for mt in range(MT):
                    ps = ppA.tile([128, 256], F32, name="pssv", tag="mm")
                    for ki in range(2):
                        nc.tensor.matmul(ps[:], styb[ki][:, mt * 128:(mt + 1) * 128],
                                         wse_sb[:, ki, :], start=(ki == 0), stop=False)
                    nc.tensor.matmul(ps[:], ones_bf[0:1, :], wse_sb[0:1, 2, :],
                                     start=False, stop=True)
                    t = bigp.tile([128, 512], BF16, name=f"sv{mt}", tag=f"sv{mt}")
                    nc.vector.tensor_copy(t[:, 0:256], ps[:])
                    nc.vector.tensor_mul(t[:, 256:512], t[:, 0:256], t[:, 0:256])
                    sv.append(t)

                # kin (inorm of style) then k-proj
                kinb = []
                for oj in range(2):
                    t = workp.tile([128, M], BF16, name=f"kin{oj}", tag="xf")
                    nc.vector.tensor_scalar(t[:], styb[oj][:], sty_rs[oj][:],
                                            sty_nb[oj][:], ALU.mult, ALU.add)
                    kinb.append(t)
                kpb = []
                for oj in range(2):
                    t = bigp.tile([128, M], BF16, name=f"kpb{oj}", tag=f"kpb{oj}")
                    for mc in range(M // NF):
                        ps = ppA.tile([128, NF], F32, name="pskp", tag="mm")
                        for ki in range(2):
                            nc.tensor.matmul(ps[:], wke_sb[:, ki, oj * 128:(oj + 1) * 128],
                                             kinb[ki][:, mc * NF:(mc + 1) * NF],
                                             start=(ki == 0), stop=(ki == 1))
                        nc.scalar.activation(t[:, mc * NF:(mc + 1) * NF], ps[:], AF.Identity,
                                             bias=bias_sb[:, B_KE + oj:B_KE + oj + 1])
                    kpb.append(t)

                # ---- c-zipper after the style phase, so the q-collective is
                # fully covered by style-side PE work ----
                c0 = zipper(wv1_sb, wv2_sb, B_VZ1, B_VZ2, "c0", 16, 24)
                stats_gc = stats_allreduce("c", 16)

                # ---- phase 3: apply collective stats; q-proj; c0 inorm ----
                q_rs, q_nb, c_rs, c_nb = [], [], [], []
                for oj in range(2):
                    rs, nb = norm_consts(stats_gq[:, oj:oj + 1],
                                         stats_gq[:, 2 + oj:3 + oj], N, f"qn{oj}")
                    q_rs.append(rs)
                    q_nb.append(nb)
                for oj in range(2):
                    rs, nb = norm_consts(stats_gc[:, oj:oj + 1],
                                         stats_gc[:, 2 + oj:3 + oj], N, f"cn{oj}")
                    c_rs.append(rs)
                    c_nb.append(nb)

                q0n = []
                for oj in range(2):
                    t = bigp.tile([128, NL], BF16, name=f"q0n{oj}", tag=f"xb{oj}")
                    nc.vector.tensor_scalar(t[:], q0[oj][:], q_rs[oj][:],
                                            q_nb[oj][:], ALU.mult, ALU.add)
                    q0n.append(t)
                qpb = [bigp.tile([128, NL], BF16, name=f"qpb{oj}", tag=f"h1_{oj}")
                       for oj in range(2)]
                for jn in range(NCH):
                    for oj in range(2):
                        ps = ppA.tile([128, NF], F32, name="psqp", tag="mm")
                        for ki in range(2):
                            nc.tensor.matmul(ps[:], wqe_sb[:, ki, oj * 128:(oj + 1) * 128],
                                             q0n[ki][:, jn * NF:(jn + 1) * NF],
                                             start=(ki == 0), stop=(ki == 1))
                        nc.scalar.activation(qpb[oj][:, jn * NF:(jn + 1) * NF], ps[:],
                                             AF.Identity,
                                             bias=bias_sb[:, B_QE + oj:B_QE + oj + 1])
                c0n = []
                for oj in range(2):
                    t = bigp.tile([128, NL], BF16, name=f"c0n{oj}", tag=f"xb{2 + oj}")
                    nc.vector.tensor_scalar(t[:], c0[oj][:], c_rs[oj][:],
                                            c_nb[oj][:], ALU.mult, ALU.add)
                    c0n.append(t)

            # ---- phase 4: attention + epilogue, software-pipelined across
            # chunks of 512 queries. The S/exp stream runs 2 tiles ahead of
            # the O accumulation and crosses chunk boundaries; each chunk's
            # output MLP is deferred into the next chunk's O-loop so the PE
            # never idles on the epilogue's DVE chain.
            with tc.tile_pool(name="ppB", bufs=1, space="PSUM") as ppB, \
                 tc.tile_pool(name="ep", bufs=2) as ep:
                eS = [[None] * MT for _ in range(NCH)]
                s_next = [0]

                def s_step():
                    g = s_next[0]
                    s_next[0] += 1
                    if g >= NCH * MT:
                        return
                    jc, mt = divmod(g, MT)
                    nsl = slice(jc * NF, (jc + 1) * NF)
                    ps = ppB.tile([128, NF], F32, name=f"pss{g}", tag="sT", bufs=3)
                    for ki in range(2):
                        nc.tensor.matmul(ps[:], kpb[ki][:, mt * 128:(mt + 1) * 128],
                                         qpb[ki][:, nsl], start=(ki == 0), stop=(ki == 1))
                    e = ep.tile([128, NF], BF16, name=f"eS{g}", tag="eS", bufs=6)
                    nc.scalar.activation(e[:], ps[:], AF.Exp)
                    eS[jc][mt] = e

                def epilogue(jc, po, zps):
                    nsl = slice(jc * NF, (jc + 1) * NF)
                    # copy po out of PSUM immediately (split across ACT and
                    # DVE) so the po slots free without waiting on the rz chain
                    osb = []
                    for g in range(4):
                        t = ep.tile([128, NF], F32, name=f"osb{jc}_{g}", tag=f"osb{g}",
                                    bufs=1)
                        if g < 2:
                            nc.scalar.copy(t[:], po[g][:])
                        else:
                            nc.vector.tensor_copy(t[:], po[g][:])
                        osb.append(t)
                    rz = ep.tile([1, NF], F32, name=f"rz{jc}", tag="rz")
                    nc.vector.reciprocal_approx_fast(rz[:], zps[:])
                    bzp = ppB.tile([128, NF], F32, name=f"bzp{jc}", tag="mlp")
                    nc.tensor.matmul(bzp[:], ones_f32[0:1, :], rz[0:1, :])
                    bz = ep.tile([128, NF], F32, name=f"bz{jc}", tag="bz", bufs=1)
                    nc.scalar.copy(bz[:], bzp[:])
                    cs = []

                    def etmp(nm):
                        return ep.tile([128, NF], F32, name=f"{nm}{jc}", tag="etmp",
                                       bufs=4)

                    for oj in range(2):
                        mean = ep.tile([128, NF], F32, name=f"mean{jc}_{oj}", tag="mean")
                        nc.vector.tensor_mul(mean[:], osb[oj][:], bz[:])
                        es2 = etmp(f"es2_{oj}_")
                        nc.vector.tensor_mul(es2[:], osb[2 + oj][:], bz[:])
                        msq = etmp(f"msq_{oj}_")
                        nc.scalar.square(msq[:], mean[:])
                        var = etmp(f"var_{oj}_")
                        nc.vector.tensor_sub(var[:], es2[:], msq[:])
                        varp = etmp(f"varp_{oj}_")
                        nc.vector.tensor_scalar_max(varp[:], var[:], 0.0)
                        std = etmp(f"std_{oj}_")
                        nc.scalar.activation(std[:], varp[:], AF.Sqrt)
                        t1 = etmp(f"t1_{oj}_")
                        nc.vector.tensor_mul(t1[:], c0n[oj][:, nsl], std[:])
                        cst = ep.tile([128, NF], BF16, name=f"cst{jc}_{oj}", tag="cst")
                        nc.vector.tensor_add(cst[:], t1[:], mean[:])
                        cs.append(cst)
                    return cs

                def make_mlp(jc, cs):
                    nsl = slice(jc * NF, (jc + 1) * NF)
                    hb = []

                    def h_step(oj):
                        ps = ppB.tile([128, NF], F32, name=f"psh{jc}_{oj}", tag="mlp")
                        for ki in range(2):
                            nc.tensor.matmul(ps[:], wu1_sb[:, ki, oj * 128:(oj + 1) * 128],
                                             cs[ki][:], start=(ki == 0), stop=(ki == 1))
                        hl = ep.tile([128, NF], BF16, name=f"hl4{jc}_{oj}", tag="hl4")
                        nc.scalar.activation(hl[:], ps[:], AF.Identity,
                                             bias=bias_sb[:, B_VU1 + oj:B_VU1 + oj + 1])
                        ht = ep.tile([128, NF], BF16, name=f"hb{jc}_{oj}", tag="hb")
                        _lrelu_dve(nc, ht[:], hl[:])
                        hb.append(ht)

                    def o_step(oc):
                        ps = ppB.tile([128, NF], F32, name=f"pso{jc}_{oc}", tag="mlp")
                        for ki in range(2):
                            nc.tensor.matmul(ps[:], wu2_sb[:, ki, oc * 128:(oc + 1) * 128],
                                             hb[ki][:], start=(ki == 0), stop=(ki == 1))
                        of = ep.tile([128, NF], F32, name=f"of{jc}_{oc}", tag="of", bufs=3)
                        nc.scalar.activation(of[:], ps[:], AF.Identity,
                                             bias=bias_sb[:, B_VU2 + oc:B_VU2 + oc + 1])
                        nc.sync.dma_start(out_d[oc * 128:(oc + 1) * 128, nsl], of[:])

                    return ([lambda oj=oj: h_step(oj) for oj in range(2)]
                            + [lambda oc=oc: o_step(oc) for oc in range(4)])

                deferred = {}
                s_step()
                s_step()
                s_step()
                for jc in range(NCH):
                    po = [ppB.tile([128, NF], F32, name=f"po{jc}_{g}", tag=f"po{g}")
                          for g in range(4)]
                    acc = None
                    for mt in range(MT):
                        s_step()
                        st, sp = (mt == 0), (mt == MT - 1)
                        for g in range(4):
                            nc.tensor.matmul(po[g][:], sv[mt][:, g * 128:(g + 1) * 128],
                                             eS[jc][mt][:], start=st, stop=sp)
                        # running sum of eS tiles on DVE (Z partial sums);
                        # last add lands in bf16 for the cheap final contraction
                        dt = BF16 if sp else F32
                        na = ep.tile([128, NF], dt, name=f"za{jc}_{mt}", tag="zacc",
                                     bufs=2)
                        if acc is None:
                            nc.vector.tensor_copy(na[:], eS[jc][mt][:])
                        else:
                            nc.vector.tensor_add(na[:], acc[:], eS[jc][mt][:])
                        acc = na
                        for fn in deferred.pop((jc, mt), []):
                            fn()
                    zps = ppB.tile([1, NF], F32, name=f"zps{jc}", tag="mlp")
                    nc.tensor.matmul(zps[:], onecol_bf[:], acc[:])
                    cs = epilogue(jc, po, zps)
                    mlp_fns = make_mlp(jc, cs)
                    if jc + 1 < NCH:
                        for idx, fn in enumerate(mlp_fns):
                            deferred.setdefault((jc + 1, 8 + idx * 3), []).append(fn)
                    else:
                        for fn in mlp_fns:
                            fn()

    nc.compile()
    return nc


def _get_nc():
    if "nc" not in _nc_cache:
        _nc_cache["nc"] = _build_nc()
    return _nc_cache["nc"]


def _prep_inputs(inputs):
    bf = ml_dtypes.bfloat16
    t = lambda a: np.ascontiguousarray(np.asarray(a).T).astype(bf)

    shared = {
        "wq1": t(inputs["qz_w1"]), "wq2": t(inputs["qz_w2"]),
        "wv1": t(inputs["vz_w1"]), "wv2": t(inputs["vz_w2"]),
        "wqe": t(inputs["qe_w"]), "wke": t(inputs["ke_w"]),
        "wu1": t(inputs["vu_w1"]), "wu2": t(inputs["vu_w2"]),
        "wse": np.vstack([np.asarray(inputs["se_w"]).T,
                          np.asarray(inputs["se_b"])[None, :]]).astype(bf),
    }
    bias = np.zeros((128, 18), np.float32)
    for col, vec in ((B_QZ1, "qz_b1"), (B_QZ2, "qz_b2"), (B_VZ1, "vz_b1"),
                     (B_VZ2, "vz_b2"), (B_QE, "qe_b"), (B_KE, "ke_b"),
                     (B_VU1, "vu_b1")):
        v = np.asarray(inputs[vec], np.float32)
        bias[:, col] = v[0:128]
        bias[:, col + 1] = v[128:256]
    v = np.asarray(inputs["vu_b2"], np.float32)
    for i in range(4):
        bias[:, B_VU2 + i] = v[i * 128:(i + 1) * 128]
    shared["bias"] = bias

    x = np.asarray(inputs["feats_in"], np.float32)
    sty = np.asarray(inputs["style_feats"], np.float32)
    in_maps = []
    for c in range(8):
        b, j = divmod(c, 4)
        m = dict(shared)
        m["x"] = np.ascontiguousarray(x[b][:, j * NL:(j + 1) * NL]).astype(bf)
        m["sty"] = np.ascontiguousarray(sty[b]).astype(bf)
        in_maps.append(m)
    return in_maps


def _run(inputs, trace=False):
    nc = _get_nc()
    in_maps = _prep_inputs(inputs)
    res = run_bass_kernel_spmd(nc, in_maps, core_ids=list(range(8)), trace=trace)
    out = np.empty((BS, C, N), np.float32)
    for c in range(8):
        b, j = divmod(c, 4)
        out[b][:, j * NL:(j + 1) * NL] = res.results[c]["out"]
    return out, res


def kernel(**inputs) -> np.ndarray:
    out, _ = _run(inputs, trace=False)
    return out
